# revision 1
# baseline (speedup 1.0000x reference)
"""Trainium2 Bass kernel for a 2-layer GRU teacher-forced decoder.

Math (per reference):
  toks[t,b]: t=0 -> SOS(=1), t>=1 -> target[b, t]   (T = ML-1 = 63 steps)
  x_t = relu(emb[toks[t]])                          [B, E]
  h0 <- GRUCell(x_t, h0; W_ih0, W_hh0, b_ih0, b_hh0)
  h1 <- GRUCell(h0, h1; W_ih1, W_hh1, b_ih1, b_hh1)
  logits_t = h1 @ W_out.T + b_out                   [B, V]
  out = stack(logits).transpose(1,0,2)              [B, T, V]

Device strategy (8 cores, SPMD, no collectives):
  - The sequential GRU recurrence is replicated on every core (it is
    PE-stream-bound, so batch-sharding would not make it any faster, and
    per-step cross-core collectives have a ~5us floor which is far too slow
    for 126 layer-steps).  Every core ends up with all H1 states.
  - The large output projection [T*B, H] @ [H, V] is sharded column-wise
    (vocab) 8 ways: core k computes logits[:, k*4000:(k+1)*4000].
  - All matmuls run in bf16 (fp32 matmul is 4 cycles/row on TRN2 PE, bf16 is
    1); gate elementwise math and state are fp32.  Host-side emulation of
    this scheme gives ~4e-3 normalized rel error.

Layouts:
  - Gate tensors live in PSUM as [128, 768] = [(quarter q, batch b), (gate, j)]
    where hidden = q*256 + j. The 4 hidden-quarters are packed into the 4
    PE column groups via tile_position=(0, 32q) so the four N=768 gate
    streams run concurrently (full 128x128 array utilization at B=32).
  - State h is kept twice: fp32 "quarter layout" [128, 256] for elementwise,
    and transposed bf16 [128(hidden chunk), 32(batch)] (as columns of a
    [128, 8, 2048] history tensor) for use as the next matmul's lhsT.
  - Gate biases are added on the Vector engine (quarter-layout fp32 bias
    tiles) on the way out of PSUM; the output-projection bias is added on
    the host after gathering the vocab shards (0.01% of the FLOPs).
  - PE matmuls on this stack can only read the stationary operand from SBUF
    base partition 0, so the state transpose stages h' quarters into a flat
    [32, 1024] tile via SBUF->SBUF DMA before the PE transposes.
"""

import os
import sys
import numpy as np

sys.path.insert(0, "/opt/trn_rl_repo")

import ml_dtypes

V, E, H, B, ML = 32000, 512, 1024, 32, 64
SOS = 1
T = ML - 1          # 63
TB = T * B          # 2016
NCORES = 8
VS = V // NCORES    # 4000 vocab slice per core
Q = 4               # hidden quarters
J = H // Q          # 256
KH = H // 128       # 8 contraction chunks over H
KE = E // 128       # 4 contraction chunks over E
MT = 126            # logits M-tile (2016 = 16 * 126)
NMT = TB // MT      # 16

_BF = ml_dtypes.bfloat16


def _bf16(x):
    return np.asarray(x, np.float32).astype(_BF)


def _prep_wT(w, kchunks):
    """w: [3H, K*128] fp32 -> [128, kchunks, 3H] bf16 with [p, k, col] = w[col, 128k+p]."""
    wt = np.ascontiguousarray(np.asarray(w, np.float32).T)       # [K, 3H]
    wt = wt.reshape(kchunks, 128, wt.shape[1]).transpose(1, 0, 2)  # [128, k, 3H]
    return _bf16(wt)


def _prep_hq(h):
    """h: [B, H] fp32 -> quarter layout [128, 256], [32q+b, j] = h[b, q*256+j]."""
    hq = np.asarray(h, np.float32).reshape(B, Q, J).transpose(1, 0, 2).reshape(Q * B, J)
    return np.ascontiguousarray(hq)


def _prep_hT(h):
    """h: [B, H] -> [128, 8, 32] bf16 with [p, k, b] = h[b, 128k+p]."""
    ht = np.asarray(h, np.float32).T.reshape(KH, 128, B).transpose(1, 0, 2)
    return _bf16(ht)


def _gate_bias_quarter(b_ih, b_hh):
    """Quarter-layout fp32 bias tiles for the DVE adds.

    bq [128, 768]: [32q+b, gate*256+j] = (b_ih+b_hh) for r,z; b_hh for n.
    bc [128, 256]: [32q+b, j] = b_ih n-part.
    """
    bi = np.asarray(b_ih, np.float32)
    bh = np.asarray(b_hh, np.float32)
    comb = np.empty(3 * H, np.float32)
    comb[0:2 * H] = bi[0:2 * H] + bh[0:2 * H]
    comb[2 * H:] = bh[2 * H:]
    bq = np.empty((128, 3 * J), np.float32)
    bc = np.empty((128, J), np.float32)
    for q in range(Q):
        s = q * J
        row = np.concatenate([comb[s:s + J], comb[H + s:H + s + J],
                              comb[2 * H + s:2 * H + s + J]])
        bq[32 * q:32 * (q + 1)] = row[None, :]
        bc[32 * q:32 * (q + 1)] = bi[2 * H + s:2 * H + s + J][None, :]
    return bq, bc


def _build_inputs(encoder_hidden, target_tensor, emb,
                  W_ih0, W_hh0, b_ih0, b_hh0, W_ih1, W_hh1, b_ih1, b_hh1,
                  W_out, b_out):
    """Host-side layout prep. Returns (shared_map, per_core_maps)."""
    tt = np.asarray(target_tensor)
    toks = np.concatenate(
        [np.full((B, 1), SOS, dtype=tt.dtype), tt[:, 1:ML - 1]], axis=1).T  # [T, B]
    X = np.maximum(np.asarray(emb, np.float32)[toks], 0.0)  # [T, B, E]
    # xT [128, KE, T*B]: [p, k, t*32+b] = X[t, b, 128k+p]
    xT = X.reshape(TB, KE, 128).transpose(2, 1, 0)
    xT = np.ascontiguousarray(_bf16(xT))

    bq0, bc0 = _gate_bias_quarter(b_ih0, b_hh0)
    bq1, bc1 = _gate_bias_quarter(b_ih1, b_hh1)

    ident = np.zeros((128, 32), np.float32)
    for g in range(4):
        ident[g * 32:(g + 1) * 32] = np.eye(32, dtype=np.float32)

    shared = {
        "xT": xT,
        "h0q": _prep_hq(encoder_hidden[0]),
        "h1q": _prep_hq(encoder_hidden[1]),
        "h0T": _prep_hT(encoder_hidden[0]),
        "h1T": _prep_hT(encoder_hidden[1]),
        "wih0T": _prep_wT(W_ih0, KE),
        "whh0T": _prep_wT(W_hh0, KH),
        "wih1T": _prep_wT(W_ih1, KH),
        "whh1T": _prep_wT(W_hh1, KH),
        "bq0": bq0, "bc0": bc0,
        "bq1": bq1, "bc1": bc1,
        "ident": ident,
    }
    wout = np.asarray(W_out, np.float32)
    bout = np.asarray(b_out, np.float32)
    per_core = []
    for c in range(NCORES):
        sl = slice(c * VS, (c + 1) * VS)
        woutT = wout[sl].T.reshape(KH, 128, VS).transpose(1, 0, 2)  # [128, 8, VS]
        per_core.append({
            "woutT": np.ascontiguousarray(_bf16(woutT)),
        })
    return shared, per_core


# ---------------------------------------------------------------------------
# Device program
# ---------------------------------------------------------------------------

def _emit(nc, tc, io, n_steps=T):
    import concourse.bass as bass
    from concourse import mybir
    from concourse.alu_op_type import AluOpType as alu

    f32 = mybir.dt.float32
    bf16 = mybir.dt.bfloat16
    Sig = mybir.ActivationFunctionType.Sigmoid
    Tanh = mybir.ActivationFunctionType.Tanh

    ctx_pools = []

    def pool(name, bufs, space="SBUF"):
        p = tc.tile_pool(name=name, bufs=bufs, space=space)
        ctx_pools.append(p)
        return p.__enter__()

    consts = pool("consts", 1)
    state = pool("state", 1)
    arena_p = pool("arena", 1)
    hqp = pool("hq", 2)
    work = pool("work", 2)
    xp = pool("xs", 3)
    stp = pool("stp", 1)

    # ---- constants / persistent tensors in SBUF ----
    ident_sb = consts.tile([128, 32], f32)
    nc.sync.dma_start(ident_sb[:], io["ident"][:])
    bq = {}
    bc = {}
    for L in (0, 1):
        bq[L] = consts.tile([128, 3 * J], f32, tag=f"bq{L}", name=f"bq{L}")
        nc.sync.dma_start(bq[L][:], io[f"bq{L}"][:])
        bc[L] = consts.tile([128, J], f32, tag=f"bc{L}", name=f"bc{L}")
        nc.sync.dma_start(bc[L][:], io[f"bc{L}"][:])

    # state history, transposed, bf16: [128, kchunk, (n_steps+1)*32]
    HT = {}
    for L in (0, 1):
        HT[L] = state.tile([128, KH, (n_steps + 1) * 32], bf16, tag=f"H{L}T", name=f"H{L}T")
        nc.sync.dma_start(HT[L][:, :, 0:32], io[f"h{L}T"][:])

    hq_init = {}
    for L in (0, 1):
        hq_init[L] = consts.tile([128, J], f32, tag=f"hq{L}i", name=f"hq{L}i")
        nc.sync.dma_start(hq_init[L][:], io[f"h{L}q"][:])

    def gate_mms_input(psum_pool, Wa, kw, lhsT_of, layer):
        """Bias seeds + input-path (W_ih) matmuls for one step.

        These only read x / the lower layer's history, so they may be
        emitted ahead of the recurrent dependency chain. Returns (G, C).
        """
        G = psum_pool.tile([128, 4 * J], f32, tag="G", name="G", bufs=3)
        for q in range(Q):
            tp = (0, 32 * q)
            Gq_rz = G[32 * q:32 * q + 32, 0:2 * J]
            Cq = G[32 * q:32 * q + 32, 3 * J:4 * J]
            # input path: gi += x_t @ W_ih.T  (rz into G, n into C)
            for k in range(kw):
                lhsT = lhsT_of(k)
                w3 = Wa[:, k, :].rearrange("p (g j) -> p g j", g=3)
                nc.tensor.matmul(Gq_rz.rearrange("p (g j) -> p g j", g=2),
                                 lhsT, w3[:, 0:2, q * J:(q + 1) * J],
                                 start=(k == 0), stop=False, tile_position=tp,
                                 skip_group_check=True)
                nc.tensor.matmul(Cq, lhsT, w3[:, 2, q * J:(q + 1) * J],
                                 start=(k == 0), stop=(k == kw - 1),
                                 tile_position=tp, skip_group_check=True)
        return G

    def gate_mms_rec(G, Wa, wofs, layer, t):
        """Recurrent-path (W_hh) matmuls; must follow the h_t state write."""
        hT = HT[layer]
        for q in range(Q):
            tp = (0, 32 * q)
            Gq_rz = G[32 * q:32 * q + 32, 0:2 * J]
            Gq_n = G[32 * q:32 * q + 32, 2 * J:3 * J]
            for k in range(KH):
                lhsT = hT[:, k, t * 32:(t + 1) * 32]
                w3 = Wa[:, wofs + k, :].rearrange("p (g j) -> p g j", g=3)
                nc.tensor.matmul(Gq_rz.rearrange("p (g j) -> p g j", g=2),
                                 lhsT, w3[:, 0:2, q * J:(q + 1) * J],
                                 start=False, stop=(k == KH - 1), tile_position=tp,
                                 skip_group_check=True)
                nc.tensor.matmul(Gq_n, lhsT, w3[:, 2, q * J:(q + 1) * J],
                                 start=(k == 0), stop=(k == KH - 1),
                                 tile_position=tp, skip_group_check=True)

    def gate_elem_update(psum_pool, G, hq_prev, layer, t):
        C = G[:, 3 * J:4 * J]
        """sigmoid/tanh + gated update; returns new hq tile; writes HT slot t+1."""
        Sp = work.tile([128, 2 * J], f32, tag="Sp")
        nc.vector.tensor_tensor(Sp[:], G[:, 0:2 * J], bq[layer][:, 0:2 * J],
                                alu.add)
        nc.scalar.activation(Sp[:, 0:J], Sp[:, 0:J], Sig)          # r (in place)
        nc.scalar.activation(Sp[:, J:2 * J], Sp[:, J:2 * J], Sig)  # z (in place)
        t0 = work.tile([128, J], f32, tag="t0")
        nc.vector.tensor_tensor(t0[:], G[:, 2 * J:3 * J], bq[layer][:, 2 * J:3 * J],
                                alu.add)
        t1 = work.tile([128, J], f32, tag="t1")
        nc.vector.tensor_tensor(t1[:], Sp[:, 0:J], t0[:], alu.mult)
        t2 = work.tile([128, J], f32, tag="t2")
        nc.vector.tensor_tensor(t2[:], C[:], bc[layer][:], alu.add)
        t3 = work.tile([128, J], f32, tag="t3")
        nc.vector.tensor_tensor(t3[:], t2[:], t1[:], alu.add)
        n_t = work.tile([128, J], f32, tag="n")
        nc.scalar.activation(n_t[:], t3[:], Tanh)
        omz = work.tile([128, J], f32, tag="omz")
        nc.vector.tensor_scalar(omz[:], Sp[:, J:2 * J], -1.0, 1.0, alu.mult, alu.add)
        zh = work.tile([128, J], f32, tag="zh")
        nc.vector.tensor_tensor(zh[:], Sp[:, J:2 * J], hq_prev[:], alu.mult)
        p_t = work.tile([128, J], f32, tag="p")
        nc.vector.tensor_tensor(p_t[:], n_t[:], omz[:], alu.mult)
        hq_new = hqp.tile([128, J], f32, tag=f"hq{layer}")
        nc.vector.tensor_tensor(hq_new[:], p_t[:], zh[:], alu.add)
        # PE matmuls can only read the stationary operand from base partition
        # 0 on this stack, so stage the quarter slices of h' into a flat
        # [32, 1024] tile via SBUF->SBUF DMA, then transpose from base 0.
        st = stp.tile([32, H], f32, tag="st", name="st")
        for q in range(Q):
            nc.sync.dma_start(st[0:32, q * J:(q + 1) * J],
                              hq_new[32 * q:32 * q + 32, :])
        # transpose h' -> bf16 [128(hidden), 32(batch)] chunks in HT slot t+1
        TP = psum_pool.tile([128, KH * 32], f32, tag="TP")
        for k in range(KH):
            nc.tensor.matmul(
                TP[:, 32 * k:32 * k + 32],
                st[0:32, 128 * k:128 * (k + 1)],
                ident_sb[0:32, :],
                is_transpose=True,
                skip_group_check=True,
            )
        nc.vector.tensor_copy(
            HT[layer][:, :, (t + 1) * 32:(t + 2) * 32],
            TP[:].rearrange("p (k b) -> p k b", k=KH),
        )
        return hq_new

    # ================= Phase R0: layer-0 recurrence =================
    xs = []

    def load_x(t):
        xt = xp.tile([128, KE, 32], bf16, tag="xt")
        nc.sync.dma_start(xt[:], io["xT"][:, :, t * 32:(t + 1) * 32])
        return xt

    with tc.tile_pool(name="psum0", bufs=2, space="PSUM") as psum0:
        a0 = arena_p.tile([128, KE + KH, 3 * H], bf16, tag="arena", name="a0")
        nc.sync.dma_start(a0[:, 0:KE, :], io["wih0T"][:])
        nc.sync.dma_start(a0[:, KE:KE + KH, :], io["whh0T"][:])
        hq_prev = hq_init[0]
        xs = [load_x(0)]
        pend = [gate_mms_input(psum0, a0, KE,
                               lambda k, x=xs[0]: x[:, k, :], 0)]
        if n_steps > 1:
            xs.append(load_x(1))
            pend.append(gate_mms_input(psum0, a0, KE,
                                       lambda k, x=xs[1]: x[:, k, :], 0))
        for t in range(n_steps):
            G = pend.pop(0)
            gate_mms_rec(G, a0, KE, 0, t)
            # prefetch two steps ahead: independent input-path MMs keep the
            # PE fed (and HAM warm) while this step's elementwise chain runs
            if t + 2 < n_steps:
                xs.append(load_x(t + 2))
                pend.append(gate_mms_input(psum0, a0, KE,
                                           lambda k, x=xs[t + 2]: x[:, k, :], 0))
            hq_prev = gate_elem_update(psum0, G, hq_prev, 0, t)

    # ================= Phase R1: layer-1 recurrence =================
    with tc.tile_pool(name="psum1", bufs=2, space="PSUM") as psum1:
        a1 = arena_p.tile([128, 2 * KH, 3 * H], bf16, tag="arena", name="a1")
        nc.sync.dma_start(a1[:, 0:KH, :], io["wih1T"][:])
        nc.sync.dma_start(a1[:, KH:2 * KH, :], io["whh1T"][:])
        hq_prev = hq_init[1]
        pend = [gate_mms_input(psum1, a1, KH,
                               lambda k: HT[0][:, k, 32:64], 1)]
        if n_steps > 1:
            pend.append(gate_mms_input(
                psum1, a1, KH, lambda k: HT[0][:, k, 64:96], 1))
        for t in range(n_steps):
            G = pend.pop(0)
            gate_mms_rec(G, a1, KH, 1, t)
            if t + 2 < n_steps:
                t2_ = t + 2
                pend.append(gate_mms_input(
                    psum1, a1, KH,
                    lambda k, tt_=t2_: HT[0][:, k, (tt_ + 1) * 32:(tt_ + 2) * 32],
                    1))
            hq_prev = gate_elem_update(psum1, G, hq_prev, 1, t)

    # ================= Phase R2: logits GEMM (vocab-sharded) =================
    NS = 500                       # psum slice width (fp32, one 2KB bank)
    with tc.tile_pool(name="psum2", bufs=2, space="PSUM") as psum2, \
         tc.tile_pool(name="outp", bufs=2) as outp:
        a2 = arena_p.tile([128, KH, VS], bf16, tag="arena", name="a2")
        nc.sync.dma_start(a2[:], io["woutT"][:])
        n_mt = (n_steps * B) // MT if (n_steps * B) % MT == 0 else 0
        if n_mt == 0:
            n_mt = (n_steps * B + MT - 1) // MT
        for m in range(n_mt):
            rows = min(MT, n_steps * B - m * MT)
            for s in range(VS // NS):
                L = psum2.tile([128, NS], f32, tag="L", name="L")
                for k in range(KH):
                    nc.tensor.matmul(
                        L[0:rows, :],
                        HT[1][:, k, 32 + m * MT:32 + m * MT + rows],
                        a2[:, k, s * NS:(s + 1) * NS],
                        start=(k == 0), stop=(k == KH - 1))
                ob = outp.tile([128, NS], f32, tag="ob", name="ob")
                nc.vector.tensor_copy(ob[0:rows, :], L[0:rows, :])
                nc.sync.dma_start(
                    io["logits"][m * MT:m * MT + rows, s * NS:(s + 1) * NS],
                    ob[0:rows, :])

    for p in reversed(ctx_pools):
        p.__exit__(None, None, None)


def _build_program(n_steps=T):
    import concourse.bacc as bacc
    import concourse.tile as tile
    from concourse import mybir

    f32 = mybir.dt.float32
    bf16 = mybir.dt.bfloat16

    nc = bacc.Bacc("TRN2", target_bir_lowering=False, debug=False,
                   num_devices=NCORES)

    def din(name, shape, dt):
        return nc.dram_tensor(name, list(shape), dt, kind="ExternalInput").ap()

    io = {
        "xT": din("xT", (128, KE, TB), bf16),
        "h0q": din("h0q", (128, J), f32),
        "h1q": din("h1q", (128, J), f32),
        "h0T": din("h0T", (128, KH, 32), bf16),
        "h1T": din("h1T", (128, KH, 32), bf16),
        "wih0T": din("wih0T", (128, KE, 3 * H), bf16),
        "whh0T": din("whh0T", (128, KH, 3 * H), bf16),
        "wih1T": din("wih1T", (128, KH, 3 * H), bf16),
        "whh1T": din("whh1T", (128, KH, 3 * H), bf16),
        "bq0": din("bq0", (128, 3 * J), f32),
        "bc0": din("bc0", (128, J), f32),
        "bq1": din("bq1", (128, 3 * J), f32),
        "bc1": din("bc1", (128, J), f32),
        "ident": din("ident", (128, 32), f32),
        "woutT": din("woutT", (128, KH, VS), bf16),
        "logits": nc.dram_tensor("logits", [TB, VS], f32,
                                 kind="ExternalOutput").ap(),
    }

    with tile.TileContext(nc) as tc:
        _emit(nc, tc, io, n_steps=n_steps)

    nc.compile()
    return nc


_CACHED = {}


def _get_program(n_steps=T):
    if n_steps not in _CACHED:
        _CACHED[n_steps] = _build_program(n_steps)
    return _CACHED[n_steps]


def kernel(encoder_outputs, encoder_hidden, target_tensor, emb,
           W_ih0, W_hh0, b_ih0, b_hh0, W_ih1, W_hh1, b_ih1, b_hh1,
           W_out, b_out, _trace=False):
    from concourse import bass_utils

    shared, per_core = _build_inputs(
        encoder_hidden, target_tensor, emb,
        W_ih0, W_hh0, b_ih0, b_hh0, W_ih1, W_hh1, b_ih1, b_hh1, W_out, b_out)

    nc = _get_program()
    in_maps = []
    for c in range(NCORES):
        m = dict(shared)
        m.update(per_core[c])
        in_maps.append(m)

    res = None
    for attempt in range(3):
        try:
            res = bass_utils.run_bass_kernel_spmd(
                nc, in_maps, core_ids=list(range(NCORES)), trace=_trace)
            break
        except Exception:
            if attempt == 2:
                raise
            import time
            time.sleep(20)

    parts = [res.results[c]["logits"].reshape(T, B, VS) for c in range(NCORES)]
    full = np.concatenate(parts, axis=2)          # [T, B, V]
    full += np.asarray(b_out, np.float32)[None, None, :]
    out = np.ascontiguousarray(full.transpose(1, 0, 2)).astype(np.float32)
    if _trace:
        kernel.last_results = res
    return out


kernel.last_results = None



# revision 5
# speedup vs baseline: 1.8693x; 1.8693x over previous
"""Trainium2 Bass kernel for a 2-layer GRU teacher-forced decoder.

Math (per reference):
  toks[t,b]: t=0 -> SOS(=1), t>=1 -> target[b, t]   (T = ML-1 = 63 steps)
  x_t = relu(emb[toks[t]])                          [B, E]
  h0 <- GRUCell(x_t, h0; W_ih0, W_hh0, b_ih0, b_hh0)
  h1 <- GRUCell(h0, h1; W_ih1, W_hh1, b_ih1, b_hh1)
  logits_t = h1 @ W_out.T + b_out                   [B, V]
  out = stack(logits).transpose(1,0,2)              [B, T, V]

Device strategy (8 cores, SPMD, no collectives):
  - The sequential GRU recurrence is replicated on every core; the large
    output projection [T*B, H] @ [H, V] is sharded column-wise (vocab)
    8 ways: core k computes logits[:, k*4000:(k+1)*4000].
  - L0's input-path gates gi0 = relu(emb[tok]) @ W_ih0.T (+ all foldable
    biases) depend only on tokens, so they are computed ON THE HOST in
    fp32, shipped as bf16, and injected into PSUM with an identity
    matmul at the start of each step's accumulation group.  Remaining
    biases enter PSUM via K=1 ones-matmuls.  The elementwise chain
    therefore has no bias adds.
  - Gate matmuls are emitted k-MAJOR so the four hidden partition-blocks
    (PE column groups, tile_position=(0,32u)) stream concurrently; the
    rz gates are accumulated before the n gate so sigmoid(r) overlaps
    the n-gate matmuls.
  - The hidden dimension is PERMUTED in the elementwise ("quarter")
    layout: partition 32u+b, free position phi holds hidden index
    128*(phi//32) + 32*u + (phi%32).  With this layout a single DVE
    stream-transpose (32x32 blocks) of the new state h' [128, 256]
    produces exactly the transposed-state layout [128, (k, b)] that the
    next step's matmul needs as its stationary operand -- no SBUF->SBUF
    staging DMA, no PE transposes.
  - All matmuls run in bf16; gate elementwise math and state are fp32.
"""

import sys
import numpy as np

sys.path.insert(0, "/opt/trn_rl_repo")

import ml_dtypes

V, E, H, B, ML = 32000, 512, 1024, 32, 64
SOS = 1
T = ML - 1          # 63
TB = T * B          # 2016
NCORES = 8
VS = V // NCORES    # 4000 vocab slice per core
U = 4               # hidden partition-blocks
PHI = 256           # free positions per block
KH = H // 128       # 8 contraction chunks over H
MT = 126            # logits M-tile (2016 = 16 * 126)
NS = 500            # logits psum slice width
GCH = 3             # gi0 steps per DMA chunk (63 = 21 * 3)

_BF = ml_dtypes.bfloat16

# hidden-index permutation: partition-block u, free phi -> hidden index
_phi = np.arange(PHI)
HIDP = (128 * (_phi[None, :] // 32) + 32 * np.arange(U)[:, None]
        + (_phi[None, :] % 32))                       # [4, 256]
COL3 = (np.arange(3)[:, None, None] * H + HIDP[None, :, :])  # [3, 4, 256]


def _bf16(x):
    return np.ascontiguousarray(np.asarray(x, np.float32).astype(_BF))


def _prep_w_perm(w):
    """w [3H, K] -> [128, K//128, 3072] bf16, col order (g, u, phi)."""
    w = np.asarray(w, np.float32)
    kc = w.shape[1] // 128
    arr = w[COL3.reshape(-1), :]                      # [3072, K]
    arr = arr.reshape(3 * H, kc, 128).transpose(2, 1, 0)
    return _bf16(arr)


def _prep_hq_perm(h):
    """h [B, H] -> [128, 256] f32: [32u+b, phi] = h[b, HIDP[u, phi]]."""
    arr = np.asarray(h, np.float32)[:, HIDP]          # [B, 4, 256]
    return np.ascontiguousarray(arr.transpose(1, 0, 2).reshape(128, PHI))


def _prep_hT(h):
    """h [B, H] -> [128, 8, 32] bf16 with [p, k, b] = h[b, 128k+p]."""
    ht = np.asarray(h, np.float32).T.reshape(KH, 128, B).transpose(1, 0, 2)
    return _bf16(ht)


def _build_inputs(encoder_hidden, target_tensor, emb,
                  W_ih0, W_hh0, b_ih0, b_hh0, W_ih1, W_hh1, b_ih1, b_hh1,
                  W_out, b_out):
    tt = np.asarray(target_tensor)
    toks = np.concatenate(
        [np.full((B, 1), SOS, dtype=tt.dtype), tt[:, 1:ML - 1]], axis=1).T
    X = np.maximum(np.asarray(emb, np.float32)[toks], 0.0)      # [T, B, E]

    bi0 = np.asarray(b_ih0, np.float32)
    bh0 = np.asarray(b_hh0, np.float32)
    bias0 = np.concatenate([bi0[:2 * H] + bh0[:2 * H], bi0[2 * H:]])
    gi0 = X.reshape(TB, E) @ np.asarray(W_ih0, np.float32).T + bias0
    gi0 = gi0[:, COL3.reshape(-1)]                    # [TB, 3072] (g,u,phi)
    gi0 = gi0.reshape(T, B, 3, U, PHI).transpose(3, 1, 0, 2, 4)
    gi0q = _bf16(gi0.reshape(128, T, 3 * PHI))        # [128, T, 768]

    bc0f = np.ascontiguousarray(bh0[2 * H:][HIDP].reshape(1, H))

    bi1 = np.asarray(b_ih1, np.float32)
    bh1 = np.asarray(b_hh1, np.float32)
    comb1 = np.concatenate([bi1[:2 * H] + bh1[:2 * H], bi1[2 * H:]])
    bq1f = np.ascontiguousarray(
        comb1[COL3].transpose(1, 0, 2).reshape(1, 3 * H))   # (u, g, phi)
    bc1f = np.ascontiguousarray(bh1[2 * H:][HIDP].reshape(1, H))

    shared = {
        "gi0q": gi0q,
        "h0q": _prep_hq_perm(encoder_hidden[0]),
        "h1q": _prep_hq_perm(encoder_hidden[1]),
        "h0T": _prep_hT(encoder_hidden[0]),
        "h1T": _prep_hT(encoder_hidden[1]),
        "whh0T": _prep_w_perm(W_hh0),
        "wih1T": _prep_w_perm(W_ih1),
        "whh1T": _prep_w_perm(W_hh1),
        "bc0f": np.asarray(bc0f, np.float32).astype(_BF),
        "bq1f": np.asarray(bq1f, np.float32).astype(_BF),
        "bc1f": np.asarray(bc1f, np.float32).astype(_BF),
        "i128": np.eye(128, dtype=np.float32).astype(_BF),
        "ones": np.ones((1, 32), np.float32).astype(_BF),
    }
    wout = np.asarray(W_out, np.float32)
    per_core = []
    for c in range(NCORES):
        sl = slice(c * VS, (c + 1) * VS)
        woutT = wout[sl].T.reshape(KH, 128, VS).transpose(1, 0, 2)
        per_core.append({"woutT": _bf16(woutT)})
    return shared, per_core


# ---------------------------------------------------------------------------
# Device program
# ---------------------------------------------------------------------------

def _emit(nc, tc, io, n_steps=T):
    from concourse import mybir
    from concourse.alu_op_type import AluOpType as alu

    f32 = mybir.dt.float32
    bf16 = mybir.dt.bfloat16
    Sig = mybir.ActivationFunctionType.Sigmoid
    Tanh = mybir.ActivationFunctionType.Tanh

    ctx_pools = []

    def pool(name, bufs, space="SBUF"):
        p = tc.tile_pool(name=name, bufs=bufs, space=space)
        ctx_pools.append(p)
        return p.__enter__()

    consts = pool("consts", 1)
    state = pool("state", 1)
    arena_p = pool("arena", 1)
    hqp = pool("hq", 2)
    work = pool("work", 2)
    gip = pool("gi", 2)

    i128 = consts.tile([128, 128], bf16)
    nc.sync.dma_start(i128[:], io["i128"][:])
    ones = consts.tile([1, 32], bf16)
    nc.sync.dma_start(ones[:], io["ones"][:])
    bc0f = consts.tile([1, H], bf16)
    nc.sync.dma_start(bc0f[:], io["bc0f"][:])
    bq1f = consts.tile([1, 3 * H], bf16)
    nc.sync.dma_start(bq1f[:], io["bq1f"][:])
    bc1f = consts.tile([1, H], bf16)
    nc.sync.dma_start(bc1f[:], io["bc1f"][:])

    HT = {}
    for L in (0, 1):
        HT[L] = state.tile([128, KH, (n_steps + 1) * 32], bf16,
                           tag=f"H{L}T", name=f"H{L}T")
        nc.sync.dma_start(HT[L][:, :, 0:32], io[f"h{L}T"][:])
    hq_init = {}
    for L in (0, 1):
        hq_init[L] = consts.tile([128, PHI], f32, tag=f"hq{L}i", name=f"hq{L}i")
        nc.sync.dma_start(hq_init[L][:], io[f"h{L}q"][:])

    def rec_mms(G, Wa, kofs, layer, t):
        """Recurrent-path matmuls, k-major, rz before n."""
        hT = HT[layer]
        for k in range(KH):
            lhsT = hT[:, k, t * 32:(t + 1) * 32]
            w3 = Wa[:, kofs + k, :].rearrange("p (g u f) -> p g u f", g=3, u=U)
            for u in range(U):
                nc.tensor.matmul(
                    G[32 * u:32 * u + 32, 0:512].rearrange(
                        "p (g j) -> p g j", g=2),
                    lhsT, w3[:, 0:2, u, :],
                    start=False, stop=(k == KH - 1),
                    tile_position=(0, 32 * u), skip_group_check=True)
        for k in range(KH):
            lhsT = hT[:, k, t * 32:(t + 1) * 32]
            w3 = Wa[:, kofs + k, :].rearrange("p (g u f) -> p g u f", g=3, u=U)
            for u in range(U):
                nc.tensor.matmul(
                    G[32 * u:32 * u + 32, 768:1024],
                    lhsT, w3[:, 2, u, :],
                    start=False, stop=(k == KH - 1),
                    tile_position=(0, 32 * u), skip_group_check=True)

    def elem(G, hq_prev, layer, t):
        """Gate nonlinearities + state update; writes HT[layer] slot t+1."""
        r = work.tile([128, PHI], f32, tag="r")
        nc.scalar.activation(r[:], G[:, 0:256], Sig)
        z = work.tile([128, PHI], f32, tag="z")
        nc.scalar.activation(z[:], G[:, 256:512], Sig)
        t1 = work.tile([128, PHI], f32, tag="t1")
        nc.vector.tensor_tensor(t1[:], r[:], G[:, 768:1024], alu.mult)
        t3 = work.tile([128, PHI], f32, tag="t3")
        nc.vector.tensor_tensor(t3[:], t1[:], G[:, 512:768], alu.add)
        omz = work.tile([128, PHI], f32, tag="omz")
        nc.vector.tensor_scalar(omz[:], z[:], -1.0, 1.0, alu.mult, alu.add)
        zh = work.tile([128, PHI], f32, tag="zh")
        nc.vector.tensor_tensor(zh[:], z[:], hq_prev[:], alu.mult)
        n_t = work.tile([128, PHI], f32, tag="n")
        nc.scalar.activation(n_t[:], t3[:], Tanh)
        p_t = work.tile([128, PHI], f32, tag="p")
        nc.vector.tensor_tensor(p_t[:], n_t[:], omz[:], alu.mult)
        hq_new = hqp.tile([128, PHI], f32, tag=f"hq{layer}")
        nc.vector.tensor_tensor(hq_new[:], p_t[:], zh[:], alu.add)
        # one DVE stream-transpose (32x32 blocks) -> canonical h^T layout
        htf = work.tile([128, PHI], f32, tag="htf")
        nc.vector.transpose(htf[:], hq_new[:])
        nc.vector.tensor_copy(
            HT[layer][:, :, (t + 1) * 32:(t + 2) * 32],
            htf[:].rearrange("p (w b) -> p w b", w=KH))
        return hq_new

    # ================= Phase R0: layer-0 recurrence =================
    with tc.tile_pool(name="psum0", bufs=3, space="PSUM") as psum0:
        a0 = arena_p.tile([128, KH, 3 * H], bf16, tag="arena", name="a0")
        nc.sync.dma_start(a0[:], io["whh0T"][:])

        n_gch = (n_steps + GCH - 1) // GCH
        gi_tiles = {}

        def load_gi(c):
            steps = min(GCH, n_steps - c * GCH)
            gt = gip.tile([128, GCH, 3 * PHI], bf16, tag="gi")
            nc.sync.dma_start(gt[:, 0:steps, :],
                              io["gi0q"][:, c * GCH:c * GCH + steps, :])
            gi_tiles[c] = gt

        load_gi(0)
        if n_gch > 1:
            load_gi(1)

        def inject0(t):
            G = psum0.tile([128, 1024], f32, tag="G", name="G")
            gt = gi_tiles[t // GCH]
            ti = t % GCH
            nc.tensor.matmul(G[:, 0:512], i128[:], gt[:, ti, 0:512],
                             start=True, stop=False, skip_group_check=True)
            nc.tensor.matmul(G[:, 512:768], i128[:], gt[:, ti, 512:768],
                             start=True, stop=True, skip_group_check=True)
            # bank B's has_written bits were cleared by the gi_n idmm
            # (start=True clears the WHOLE 2KB bank); these must accumulate
            # semantics: unset bits -> overwrite, so start=False is correct.
            for u in range(U):
                nc.tensor.matmul(
                    G[32 * u:32 * u + 32, 768:1024],
                    ones[0:1, :], bc0f[0:1, u * PHI:(u + 1) * PHI],
                    start=False, stop=False,
                    tile_position=(0, 32 * u), skip_group_check=True)
            return G

        pend = [inject0(0)]
        if n_steps > 1:
            pend.append(inject0(1))
        hq_prev = hq_init[0]
        for t in range(n_steps):
            G = pend.pop(0)
            rec_mms(G, a0, 0, 0, t)
            if t + 2 < n_steps:
                c = (t + 2) // GCH
                if c not in gi_tiles:
                    load_gi(c)
                pend.append(inject0(t + 2))
            hq_prev = elem(G, hq_prev, 0, t)

    # ================= Phase R1: layer-1 recurrence =================
    with tc.tile_pool(name="psum1", bufs=3, space="PSUM") as psum1:
        a1 = arena_p.tile([128, 2 * KH, 3 * H], bf16, tag="arena", name="a1")
        nc.sync.dma_start(a1[:, 0:KH, :], io["wih1T"][:])
        nc.sync.dma_start(a1[:, KH:2 * KH, :], io["whh1T"][:])

        def inject1(t):
            G = psum1.tile([128, 1024], f32, tag="G", name="G")
            for u in range(U):
                nc.tensor.matmul(
                    G[32 * u:32 * u + 32, 0:512],
                    ones[0:1, :], bq1f[0:1, u * 768:u * 768 + 512],
                    start=True, stop=False,
                    tile_position=(0, 32 * u), skip_group_check=True)
                nc.tensor.matmul(
                    G[32 * u:32 * u + 32, 512:768],
                    ones[0:1, :], bq1f[0:1, u * 768 + 512:(u + 1) * 768],
                    start=True, stop=False,
                    tile_position=(0, 32 * u), skip_group_check=True)
                nc.tensor.matmul(
                    G[32 * u:32 * u + 32, 768:1024],
                    ones[0:1, :], bc1f[0:1, u * PHI:(u + 1) * PHI],
                    start=False, stop=False,
                    tile_position=(0, 32 * u), skip_group_check=True)
            # input path gi1 = h0(t) @ W_ih1.T  (k-major, rz then n)
            for k in range(KH):
                lhsT = HT[0][:, k, (t + 1) * 32:(t + 2) * 32]
                w3 = a1[:, k, :].rearrange("p (g u f) -> p g u f", g=3, u=U)
                for u in range(U):
                    nc.tensor.matmul(
                        G[32 * u:32 * u + 32, 0:512].rearrange(
                            "p (g j) -> p g j", g=2),
                        lhsT, w3[:, 0:2, u, :],
                        start=False, stop=False,
                        tile_position=(0, 32 * u), skip_group_check=True)
            for k in range(KH):
                lhsT = HT[0][:, k, (t + 1) * 32:(t + 2) * 32]
                w3 = a1[:, k, :].rearrange("p (g u f) -> p g u f", g=3, u=U)
                for u in range(U):
                    nc.tensor.matmul(
                        G[32 * u:32 * u + 32, 512:768],
                        lhsT, w3[:, 2, u, :],
                        start=False, stop=(k == KH - 1),
                        tile_position=(0, 32 * u), skip_group_check=True)
            return G

        pend = [inject1(0)]
        if n_steps > 1:
            pend.append(inject1(1))
        hq_prev = hq_init[1]
        for t in range(n_steps):
            G = pend.pop(0)
            rec_mms(G, a1, KH, 1, t)
            if t + 2 < n_steps:
                pend.append(inject1(t + 2))
            hq_prev = elem(G, hq_prev, 1, t)

    # ================= Phase R2: logits GEMM (vocab-sharded) =================
    with tc.tile_pool(name="psum2", bufs=2, space="PSUM") as psum2, \
         tc.tile_pool(name="outp", bufs=2) as outp:
        a2 = arena_p.tile([128, KH, VS], bf16, tag="arena", name="a2")
        nc.sync.dma_start(a2[:], io["woutT"][:])
        n_mt = (n_steps * B + MT - 1) // MT
        for m in range(n_mt):
            rows = min(MT, n_steps * B - m * MT)
            for s in range(VS // NS):
                L = psum2.tile([128, NS], f32, tag="L", name="L")
                for k in range(KH):
                    nc.tensor.matmul(
                        L[0:rows, :],
                        HT[1][:, k, 32 + m * MT:32 + m * MT + rows],
                        a2[:, k, s * NS:(s + 1) * NS],
                        start=(k == 0), stop=(k == KH - 1))
                ob = outp.tile([128, NS], f32, tag="ob", name="ob")
                nc.vector.tensor_copy(ob[0:rows, :], L[0:rows, :])
                nc.sync.dma_start(
                    io["logits"][m * MT:m * MT + rows, s * NS:(s + 1) * NS],
                    ob[0:rows, :])

    for p in reversed(ctx_pools):
        p.__exit__(None, None, None)


def _build_program(n_steps=T):
    import concourse.bacc as bacc
    import concourse.tile as tile
    from concourse import mybir

    f32 = mybir.dt.float32
    bf16 = mybir.dt.bfloat16

    nc = bacc.Bacc("TRN2", target_bir_lowering=False, debug=False,
                   num_devices=NCORES)

    def din(name, shape, dt):
        return nc.dram_tensor(name, list(shape), dt, kind="ExternalInput").ap()

    io = {
        "gi0q": din("gi0q", (128, T, 3 * PHI), bf16),
        "h0q": din("h0q", (128, PHI), f32),
        "h1q": din("h1q", (128, PHI), f32),
        "h0T": din("h0T", (128, KH, 32), bf16),
        "h1T": din("h1T", (128, KH, 32), bf16),
        "whh0T": din("whh0T", (128, KH, 3 * H), bf16),
        "wih1T": din("wih1T", (128, KH, 3 * H), bf16),
        "whh1T": din("whh1T", (128, KH, 3 * H), bf16),
        "bc0f": din("bc0f", (1, H), bf16),
        "bq1f": din("bq1f", (1, 3 * H), bf16),
        "bc1f": din("bc1f", (1, H), bf16),
        "i128": din("i128", (128, 128), bf16),
        "ones": din("ones", (1, 32), bf16),
        "woutT": din("woutT", (128, KH, VS), bf16),
        "logits": nc.dram_tensor("logits", [TB, VS], f32,
                                 kind="ExternalOutput").ap(),
    }

    with tile.TileContext(nc) as tc:
        _emit(nc, tc, io, n_steps=n_steps)

    nc.compile()
    return nc


_CACHED = {}


def _get_program(n_steps=T):
    if n_steps not in _CACHED:
        _CACHED[n_steps] = _build_program(n_steps)
    return _CACHED[n_steps]


def kernel(encoder_outputs, encoder_hidden, target_tensor, emb,
           W_ih0, W_hh0, b_ih0, b_hh0, W_ih1, W_hh1, b_ih1, b_hh1,
           W_out, b_out, _trace=False):
    from concourse import bass_utils

    shared, per_core = _build_inputs(
        encoder_hidden, target_tensor, emb,
        W_ih0, W_hh0, b_ih0, b_hh0, W_ih1, W_hh1, b_ih1, b_hh1, W_out, b_out)

    nc = _get_program()
    in_maps = []
    for c in range(NCORES):
        m = dict(shared)
        m.update(per_core[c])
        in_maps.append(m)

    res = None
    for attempt in range(3):
        try:
            res = bass_utils.run_bass_kernel_spmd(
                nc, in_maps, core_ids=list(range(NCORES)), trace=_trace)
            break
        except Exception:
            if attempt == 2:
                raise
            import time
            time.sleep(20)

    parts = [res.results[c]["logits"].reshape(T, B, VS) for c in range(NCORES)]
    full = np.concatenate(parts, axis=2)          # [T, B, V]
    full += np.asarray(b_out, np.float32)[None, None, :]
    out = np.ascontiguousarray(full.transpose(1, 0, 2)).astype(np.float32)
    if _trace:
        kernel.last_results = res
    return out


kernel.last_results = None
